# revision 1
# baseline (speedup 1.0000x reference)
"""Trainium2 Bass kernel for nn_DiscreteExactLoss (joint-entropy loss).

Reference computation:
    soft_assign[b, r, :] = [1 - a[b,r], a[b,r]]          (K=2, R=10)
    joint_p[b, s]  = prod_r soft_assign[b, r, s_r]       (s in [0, 1024))
    p_a            = mean_b joint_p                       [1024]
    out            = sum_s p_a * log2(p_a)               (scalar, ~-10)

Device algorithm (per core, data-parallel over B across 8 cores):
    Instead of joint probabilities we accumulate MULTILINEAR MOMENTS
    m_T = sum_b prod_{r in T} a[b, r] for all 1024 subsets T. Moments
    factor over a 5+5 variable split: m_{T1 u T2} = sum_b MA[b,T1]*MC[b,T2],
    where MA/MC are the 32 subset-products of each 5-var half (built with
    only 30 bf16 multiplies/sample on DVE thanks to an implicit ones
    column), and the 32x32 outer product + sum over b is a TensorEngine
    matmul accumulated in PSUM (contraction = 128 samples per matmul).

    Host side: sum the 8 per-core [32,32] moment partials, apply the
    tiny Mobius transform (moments -> probabilities, 10 butterfly
    stages over a 1024-vector), then p*log2(p) reduction. This is the
    "gather/unshard" step: ~30k flops total, negligible vs. any on-device
    collective (small-message AllReduce has a ~20us latency floor).
"""

import math
import sys

import numpy as np

if "/opt/trn_rl_repo" not in sys.path:
    sys.path.insert(0, "/opt/trn_rl_repo")

B_FULL = 131072
R_FULL = 10
N_CORES = 8
B_LOC = B_FULL // N_CORES  # 16384
P = 128                    # SBUF partitions; samples per matmul chunk
C = B_LOC // P             # 128 samples per partition
NBLK = 4                   # c-blocks per core
BC = C // NBLK             # 32 samples-per-partition per block

_NC_CACHE = {}


def _build_module():
    if "nc" in _NC_CACHE:
        return _NC_CACHE["nc"]

    from concourse import bacc, bass, mybir, tile

    f32 = mybir.dt.float32
    bf16 = mybir.dt.bfloat16

    nc = bacc.Bacc("TRN2", target_bir_lowering=False, debug=False)

    act = nc.dram_tensor("act", [B_LOC, R_FULL], f32, kind="ExternalInput")
    msum = nc.dram_tensor("msum", [32, 32], f32, kind="ExternalOutput")

    # dram view [p, c, r]: sample b = p*C + c
    act_pcr = act.ap().rearrange("(p c) r -> p c r", p=P)

    with tile.TileContext(nc) as tc:
        with (
            tc.tile_pool(name="a0", bufs=3) as a0_pool,
            tc.tile_pool(name="abf", bufs=3) as abf_pool,
            tc.tile_pool(name="mac", bufs=2) as mac_pool,
            tc.tile_pool(name="outp", bufs=1) as out_pool,
            tc.tile_pool(name="psum", bufs=1, space=bass.MemorySpace.PSUM) as psum_pool,
        ):
            psum_acc = psum_pool.tile([32, 32], f32)

            for blk in range(NBLK):
                # ---- load raw fp32 activity for this block of samples ----
                a0 = a0_pool.tile([P, BC, R_FULL], f32, tag="a0")
                nc.sync.dma_start(
                    out=a0[:, :, :],
                    in_=act_pcr[:, blk * BC:(blk + 1) * BC, :],
                )

                # ---- cast to bf16, plane layout [l, h, cc] ----
                # plane q = l*2 + h holds var (l + 5h); cc innermost.
                abf = abf_pool.tile([P, 5, 2, BC], bf16, tag="abf")
                # in: dims (l, h, cc) with strides (1, 5, 10) over act's r/c
                a0_lhc = a0.rearrange("p c (h l) -> p l h c", h=2)
                nc.vector.tensor_copy(abf[:, :, :, :], a0_lhc)

                # ---- build subset-product tables for both halves ----
                # mac[p, h, m, cc]: m = 5-bit subset mask of half h's vars
                mac = mac_pool.tile([P, 2, 32, BC], bf16, tag="mac")
                # m=0 column := 1.0 (empty product)
                nc.vector.memset(mac[:, :, 0:1, :], 1.0)
                # m=1 column := var 0 of each half  (planes l=0)
                nc.vector.tensor_copy(mac[:, :, 1:2, :], abf[:, 0:1, :, :].rearrange("p l h c -> p h l c"))
                # level l: m in [2^l, 2^(l+1)) := m' in [0, 2^l) * a_{var l}
                for lvl in range(1, 5):
                    j = 1 << lvl
                    a_bcast = abf[:, lvl, :, :].unsqueeze(2).broadcast_to([P, 2, j, BC])
                    nc.vector.tensor_tensor(
                        mac[:, :, j:2 * j, :],
                        mac[:, :, 0:j, :],
                        a_bcast,
                        mybir.AluOpType.mult,
                    )

                # ---- accumulate sum_b MA (x) MC on the TensorEngine ----
                for cc in range(BC):
                    cg = blk * BC + cc
                    nc.tensor.matmul(
                        psum_acc[:, :],
                        mac[:, 0, :, cc],   # lhsT [K=128, M=32]
                        mac[:, 1, :, cc],   # rhs  [K=128, N=32]
                        start=(cg == 0),
                        stop=(cg == C - 1),
                    )

            out_sb = out_pool.tile([32, 32], f32)
            nc.vector.tensor_copy(out_sb[:, :], psum_acc[:, :])
            nc.sync.dma_start(out=msum[:, :], in_=out_sb[:, :])

    # Bacc modules carry virtual registers until compile() runs; the
    # bass2jax/PJRT path serializes nc as-is, so allocate them now.
    nc.compile()
    _NC_CACHE["nc"] = nc
    return nc


def _ensure_ntff_hook():
    """The agent image's antenv package lacks axon_hooks; synthesize it so
    run_bass_kernel_spmd(trace=True) can find the NTFF profile hook."""
    import types

    try:
        from antenv.axon_hooks import get_axon_ntff_profile_hook  # noqa: F401
        return
    except ImportError:
        pass
    import antenv

    mod = types.ModuleType("antenv.axon_hooks")
    state = {"hook": None}
    mod.set_axon_ntff_profile_hook = lambda h: state.__setitem__("hook", h)
    mod.get_axon_ntff_profile_hook = lambda: state["hook"]
    antenv.axon_hooks = mod
    sys.modules["antenv.axon_hooks"] = mod

    try:
        from trn_agent_boot.trn_boot import _ntff_profile_via_ctypes

        hook = _ntff_profile_via_ctypes("/opt/axon/libaxon_pjrt.so")
        if hook is not None:
            mod.set_axon_ntff_profile_hook(hook)
    except Exception:
        pass


def _run_on_device(activity, trace=False):
    from concourse.bass_utils import run_bass_kernel_spmd

    if trace:
        _ensure_ntff_hook()
    nc = _build_module()
    shards = np.ascontiguousarray(activity.astype(np.float32)).reshape(
        N_CORES, B_LOC, R_FULL
    )
    in_maps = [{"act": np.ascontiguousarray(shards[i])} for i in range(N_CORES)]
    res = run_bass_kernel_spmd(
        nc, in_maps, core_ids=list(range(N_CORES)), trace=trace
    )
    return res


def _finish_on_host(per_core_msums):
    # total moment sums over all B samples
    msum = np.zeros((32, 32), dtype=np.float64)
    for part in per_core_msums:
        msum += part.astype(np.float64)
    m = (msum / B_FULL).reshape(-1)  # [1024] mean moments

    # Mobius transform per bit: p(bit=0) = m(without) - m(with)
    p = m.copy()
    idx = np.arange(1024)
    for bit in range(10):
        step = 1 << bit
        lo = idx[(idx & step) == 0]
        p[lo] = p[lo] - p[lo | step]

    p = p.astype(np.float32)
    p_safe = np.clip(p, 1e-12, None)
    log_k_p = np.log(p_safe) / math.log(2.0)
    joint_h = -np.sum(p * log_k_p)
    return np.array(-joint_h, dtype=np.float32)


def kernel(activity):
    res = _run_on_device(activity, trace=False)
    return _finish_on_host([r["msum"] for r in res.results])


def kernel_profiled(activity):
    """Like kernel() but with NTFF tracing; returns (output, exec_time_ns)."""
    res = _run_on_device(activity, trace=True)
    out = _finish_on_host([r["msum"] for r in res.results])
    return out, res.exec_time_ns



# revision 3
# speedup vs baseline: 1.1042x; 1.1042x over previous
"""Trainium2 Bass kernel for nn_DiscreteExactLoss (joint-entropy loss).

Reference computation:
    soft_assign[b, r, :] = [1 - a[b,r], a[b,r]]          (K=2, R=10)
    joint_p[b, s]  = prod_r soft_assign[b, r, s_r]       (s in [0, 1024))
    p_a            = mean_b joint_p                       [1024]
    out            = sum_s p_a * log2(p_a)               (scalar, ~-10)

Device algorithm (per core, data-parallel over B across 8 cores):
    Accumulate MULTILINEAR MOMENTS m_T = sum_b prod_{r in T} a[b, r] for
    all 1024 subsets T via a 5+5 variable split: per sample, two 32-entry
    subset-product tables (A half = vars 0-4, C half = vars 5-9) built by
    doubling on DVE; cross moments = sum_b MA[b] (x) MC[b] on the
    TensorEngine with contraction over samples.

    v2 layout: samples are organized as 128 chunks of 128 (partition =
    position within chunk). Four chunks are packed per matmul: lhsT/rhs
    are [128, 128] (cols = 32 table entries x 4 chunks), so there are
    only 32 LDWEIGHTS+MATMUL pairs (vs 128 tiny ones), the weights are
    contiguous 128-col bf16 (FWL-eligible), and N=128 streaming per MM.
    The [128,128] PSUM accumulator holds a 4x4 grid of 32x32 blocks of
    which only the 4 diagonal ones (cg==cg') are meaningful; the host
    extracts and sums them. A handful of dummy warm-up matmuls run during
    the initial DMA window to flip the PE HAM clock-gate to full rate.

    Per-tile pipeline over 4 tiles of 32 chunks: DMA (HWDGE, 164KB) ->
    ACT cast+transpose to chunk-innermost bf16 -> DVE doubling (2x-mode
    tensor_tensor) -> PE matmuls.

    Host side: sum the 8 per-core partials, apply the tiny Mobius
    transform (moments -> probabilities, 10 butterfly stages over a
    1024-vector), then p*log2(p) reduction (~30k flops, negligible).
"""

import math
import sys

import numpy as np

if "/opt/trn_rl_repo" not in sys.path:
    sys.path.insert(0, "/opt/trn_rl_repo")

B_FULL = 131072
R_FULL = 10
N_CORES = 8
B_LOC = B_FULL // N_CORES  # 16384
P = 128                    # SBUF partitions; samples per chunk
C = B_LOC // P             # 128 chunks per core
NTILE = 4                  # pipeline tiles
TC = C // NTILE            # 32 chunks per tile
NG = TC // 4               # 8 matmul groups per tile (4 chunks each)
N_WARM = 12                # PE warm-up matmuls

_NC_CACHE = {}


def _build_module():
    if "nc" in _NC_CACHE:
        return _NC_CACHE["nc"]

    from concourse import bacc, bass, mybir, tile

    f32 = mybir.dt.float32
    bf16 = mybir.dt.bfloat16

    nc = bacc.Bacc("TRN2", target_bir_lowering=False, debug=False)

    act = nc.dram_tensor("act", [B_LOC, R_FULL], f32, kind="ExternalInput")
    msum = nc.dram_tensor("msum", [P, P], f32, kind="ExternalOutput")

    # dram view [p, c, r]: sample b = p*C + c
    act_pcr = act.ap().rearrange("(p c) r -> p c r", p=P)

    with tile.TileContext(nc) as tc:
        with (
            tc.tile_pool(name="a0", bufs=4) as a0_pool,
            tc.tile_pool(name="avar", bufs=2) as avar_pool,
            tc.tile_pool(name="tab", bufs=2) as tab_pool,
            tc.tile_pool(name="warm", bufs=1) as warm_pool,
            tc.tile_pool(name="outp", bufs=1) as out_pool,
            tc.tile_pool(name="psum", bufs=1, space=bass.MemorySpace.PSUM) as psum_pool,
            tc.tile_pool(name="psumw", bufs=1, space=bass.MemorySpace.PSUM) as psumw_pool,
        ):
            psum_acc = psum_pool.tile([P, P], f32)

            # ---- PE warm-up: dummy matmuls during the DMA window trip the
            # HAM activity monitor so the real matmuls run at 2.4 GHz.
            warm_sb = warm_pool.tile([P, P], bf16)
            warm_ps = psumw_pool.tile([P, P], f32)
            nc.vector.memset(warm_sb[:, :], 0.0)
            for _ in range(N_WARM):
                nc.tensor.matmul(
                    warm_ps[:, :], warm_sb[:, :], warm_sb[:, :],
                    start=True, stop=True,
                )

            for t in range(NTILE):
                # ---- load raw fp32 activity for this tile's chunks ----
                a0 = a0_pool.tile([P, TC, R_FULL], f32, tag="a0")
                nc.sync.dma_start(
                    out=a0[:, :, :],
                    in_=act_pcr[:, t * TC:(t + 1) * TC, :],
                )

                # ---- ACT: cast f32->bf16 + transpose to chunk-innermost ----
                # avar[p, l, h, c]: var r = h*5 + l of chunk c (within tile)
                avar = avar_pool.tile([P, 5, 2, TC], bf16, tag="avar")
                nc.scalar.copy(
                    avar[:, :, :, :],
                    a0.rearrange("p c (h l) -> p l h c", h=2),
                )

                # ---- DVE: subset-product tables by doubling ----
                # tab[p, h, g, i, cg]: i = 5-bit subset of half h's vars,
                # for chunk (t*8+g)*4+cg. cg innermost => 2x TT mode; ops
                # use (h g)-merged 3-free-dim APs (ISA limit).
                tab = tab_pool.tile([P, 2, NG, 32, 4], bf16, tag="tab")

                def tabv(lo, hi):
                    return tab[:, :, :, lo:hi, :].rearrange(
                        "p h g i c -> p (h g) i c"
                    )

                def avarv(lvl):
                    return avar[:, lvl, :, :].rearrange(
                        "p h (g c) -> p (h g) c", g=NG
                    ).unsqueeze(2)

                nc.vector.memset(tabv(0, 1), 1.0)
                nc.vector.tensor_copy(tabv(1, 2), avarv(0))
                for lvl in range(1, 5):
                    j = 1 << lvl
                    nc.vector.tensor_tensor(
                        tabv(j, 2 * j),
                        tabv(0, j),
                        avarv(lvl).broadcast_to([P, 2 * NG, j, 4]),
                        mybir.AluOpType.mult,
                    )

                # ---- PE: accumulate sum_b MA (x) MC, 4 chunks per MM ----
                for g in range(NG):
                    gg = t * NG + g
                    nc.tensor.matmul(
                        psum_acc[:, :],
                        tab[:, 0, g, :, :].rearrange("p i c -> p (i c)"),
                        tab[:, 1, g, :, :].rearrange("p i c -> p (i c)"),
                        start=(gg == 0),
                        stop=(gg == NTILE * NG - 1),
                    )

            out_sb = out_pool.tile([P, P], f32)
            nc.scalar.copy(out_sb[:, :], psum_acc[:, :])
            nc.sync.dma_start(out=msum[:, :], in_=out_sb[:, :])

    # Bacc modules carry virtual registers until compile() runs; the
    # bass2jax/PJRT path serializes nc as-is, so allocate them now.
    nc.compile()
    _NC_CACHE["nc"] = nc
    return nc


def _ensure_ntff_hook():
    """The agent image's antenv package lacks axon_hooks; synthesize it so
    run_bass_kernel_spmd(trace=True) can find the NTFF profile hook."""
    import types

    try:
        from antenv.axon_hooks import get_axon_ntff_profile_hook  # noqa: F401
        return
    except ImportError:
        pass
    import antenv

    mod = types.ModuleType("antenv.axon_hooks")
    state = {"hook": None}
    mod.set_axon_ntff_profile_hook = lambda h: state.__setitem__("hook", h)
    mod.get_axon_ntff_profile_hook = lambda: state["hook"]
    antenv.axon_hooks = mod
    sys.modules["antenv.axon_hooks"] = mod

    try:
        from trn_agent_boot.trn_boot import _ntff_profile_via_ctypes

        hook = _ntff_profile_via_ctypes("/opt/axon/libaxon_pjrt.so")
        if hook is not None:
            mod.set_axon_ntff_profile_hook(hook)
    except Exception:
        pass


def _run_on_device(activity, trace=False):
    from concourse.bass_utils import run_bass_kernel_spmd

    if trace:
        _ensure_ntff_hook()
    nc = _build_module()
    shards = np.ascontiguousarray(activity.astype(np.float32)).reshape(
        N_CORES, B_LOC, R_FULL
    )
    in_maps = [{"act": np.ascontiguousarray(shards[i])} for i in range(N_CORES)]
    res = run_bass_kernel_spmd(
        nc, in_maps, core_ids=list(range(N_CORES)), trace=trace
    )
    return res


def _finish_on_host(per_core_msums):
    # total moment sums over all B samples; psum is a 4x4 grid of 32x32
    # blocks (m=(i,cg), n=(j,cg')) of which the cg==cg' diagonal holds
    # per-chunk-group moment partials.
    acc = np.zeros((P, P), dtype=np.float64)
    for part in per_core_msums:
        acc += part.astype(np.float64)
    p4 = acc.reshape(32, 4, 32, 4)
    msum = sum(p4[:, k, :, k] for k in range(4))
    m = (msum / B_FULL).reshape(-1)  # [1024] mean moments

    # Mobius transform per bit: p(bit=0) = m(without) - m(with)
    p = m.copy()
    idx = np.arange(1024)
    for bit in range(10):
        step = 1 << bit
        lo = idx[(idx & step) == 0]
        p[lo] = p[lo] - p[lo | step]

    p = p.astype(np.float32)
    p_safe = np.clip(p, 1e-12, None)
    log_k_p = np.log(p_safe) / math.log(2.0)
    joint_h = -np.sum(p * log_k_p)
    return np.array(-joint_h, dtype=np.float32)


def kernel(activity):
    res = _run_on_device(activity, trace=False)
    return _finish_on_host([r["msum"] for r in res.results])


def kernel_profiled(activity):
    """Like kernel() but with NTFF tracing; returns (output, exec_time_ns)."""
    res = _run_on_device(activity, trace=True)
    out = _finish_on_host([r["msum"] for r in res.results])
    return out, res.exec_time_ns
